# revision 5
# baseline (speedup 1.0000x reference)
"""Trainium2 Bass kernel for nn_Critic (MLP preamble + 127-step LSTM + complex head).

Sharding: pure data-parallel over batch. B=256 -> 8 cores x 32. All weights
replicated; no collectives. Each core returns its [32, 512] output slice and
the host concatenates.

On-chip layout is "transposed" (feature-on-partition) end to end:
    h^T, c^T : [128 (hid%128), 4 (hid//128), 32 (batch)]
    z^T      : [128 (gate%128), 16 (gate//128), 32 (batch)]
so elementwise ops use all 128 lanes and h^T feeds the next step's matmul
(rhs) without any per-step transpose. All matmuls are bf16 inputs with fp32
PSUM accumulation (measured rel-L2 vs fp32 reference ~4e-3).

x@Wx + b_lstm for all 127 steps is hoisted out of the scan and computed as
one big parallel matmul into SBUF (bf16, ~127KB/partition); the sequential
recurrence then only does z = Wh^T h + X[t] per step.
"""

import numpy as np

import concourse.bass as bass  # noqa: F401
import concourse.mybir as mybir
from concourse import bacc, masks, tile
from concourse.bass_utils import run_bass_kernel_spmd

dt = mybir.dt
AF = mybir.ActivationFunctionType

B = 32          # batch per core
NCORES = 8
T = 127         # scan steps (63 history + 64 action)
F = 256         # lstm input feature dim
H = 512         # lstm hidden
G = 2048        # gate dim
NM = G // 128   # 16 gate tiles
KH = H // 128   # 4 hidden chunks
KX = F // 128   # 2 input chunks
NTB = T * B     # 4064 flattened (t, b)
NSL = 8         # n-slices for precompute
SL = NTB // NSL  # 508 columns per slice

PER_BATCH = ("motion_state", "robot_state", "osc_state", "action", "osc", "history")
SHAPES = {
    "motion_state": (B, 128), "robot_state": (B, 64), "osc_state": (B, 128),
    "action": (B, 64, 256), "osc": (B, 128), "history": (B, 64, 256),
    "W_ros": (128, 256), "b_ros": (256,), "W_ios": (128, 256), "b_ios": (256,),
    "W_cos": (512, 256), "b_cos": (256,), "W_ms": (128, 256), "b_ms": (256,),
    "W_rs": (64, 256), "b_rs": (256,), "W_c": (768, 512), "b_c": (512,),
    "Wx": (256, 2048), "Wh": (512, 2048), "b_lstm": (2048,),
    "Wr_out": (256, 256), "Wi_out": (256, 256), "br_out": (256,), "bi_out": (256,),
}


def _bias_t(nc, pool, src_ap, n):
    """DMA a [n*128] bias vector into a [128, n] tile (partition = dim%128)."""
    t = pool.tile([128, n], dt.float32, name=f"bias_{src_ap.tensor.name}")
    nc.sync.dma_start(t[:, :], src_ap.rearrange("(j p) -> p j", p=128))
    return t


def build(tc, outs, ins, n_steps=T):
    nc = tc.nc
    f32, bf16 = dt.float32, dt.bfloat16
    Sig, Tanh = AF.Sigmoid, AF.Tanh

    import contextlib
    ctx = contextlib.ExitStack()
    with ctx:
        # ---------------- pools ----------------
        const = ctx.enter_context(tc.tile_pool(name="const", bufs=1))
        rec = ctx.enter_context(tc.tile_pool(name="rec", bufs=2))
        pz = ctx.enter_context(tc.tile_pool(name="pz", bufs=2, space="PSUM"))
        ptr = ctx.enter_context(tc.tile_pool(name="ptr", bufs=2, space="PSUM"))

        # identity via iota(j - p) == 0 — avoids affine_select's register
        # fill, which walrus fails to allocate on this compile path
        ident = const.tile([128, 128], f32, name="ident")
        iota_t = const.tile([128, 128], dt.int32, name="iota_t")
        nc.gpsimd.iota(iota_t[:, :], pattern=[[1, 128]], base=0,
                       channel_multiplier=-1)
        nc.vector.tensor_scalar(ident[:, :], iota_t[:, :], 0, None,
                                mybir.AluOpType.is_equal)

        # persistent weights / state
        Wh_bf = const.tile([128, KH, G], bf16, name="Wh_bf")
        Wr_bf = const.tile([128, 2, 256], bf16, name="Wr_bf")
        Wi_bf = const.tile([128, 2, 256], bf16, name="Wi_bf")
        Wineg_bf = const.tile([128, 2, 256], bf16, name="Wineg_bf")
        XT = const.tile([128, NM, n_steps, B], bf16, name="XT")
        c_st = const.tile([128, KH, B], f32, name="c_st")

        b_lstm_t = _bias_t(nc, const, ins["b_lstm"], NM)
        b_ros_t = _bias_t(nc, const, ins["b_ros"], 2)
        b_ios_t = _bias_t(nc, const, ins["b_ios"], 2)
        b_cos_t = _bias_t(nc, const, ins["b_cos"], 2)
        b_ms_t = _bias_t(nc, const, ins["b_ms"], 2)
        b_rs_t = _bias_t(nc, const, ins["b_rs"], 2)
        b_c_t = _bias_t(nc, const, ins["b_c"], 4)
        br_t = _bias_t(nc, const, ins["br_out"], 2)
        bi_t = _bias_t(nc, const, ins["bi_out"], 2)

        # ============ phase A+B+C: weights, seq transpose, X precompute ======
        with tc.tile_pool(name="pre", bufs=1) as pre, \
             tc.tile_pool(name="wload", bufs=2) as wload, \
             tc.tile_pool(name="seqload", bufs=3) as seqload:

            # ---- A: big weights -> bf16 sbuf
            Wx_bf = pre.tile([128, KX, G], bf16, name="Wx_bf")
            for k in range(KH):
                for h2 in range(2):
                    wtmp = wload.tile([128, 1024], f32, tag="wl")
                    nc.sync.dma_start(
                        wtmp[:, :], ins["Wh"][k * 128:(k + 1) * 128,
                                              h2 * 1024:(h2 + 1) * 1024])
                    nc.any.tensor_copy(
                        Wh_bf[:, k, h2 * 1024:(h2 + 1) * 1024], wtmp[:, :])
            for k in range(KX):
                for h2 in range(2):
                    wtmp = wload.tile([128, 1024], f32, tag="wl")
                    nc.sync.dma_start(
                        wtmp[:, :], ins["Wx"][k * 128:(k + 1) * 128,
                                              h2 * 1024:(h2 + 1) * 1024])
                    nc.any.tensor_copy(
                        Wx_bf[:, k, h2 * 1024:(h2 + 1) * 1024], wtmp[:, :])
            for k in range(2):
                wtmp = wload.tile([128, 1024], f32, tag="wl")
                nc.sync.dma_start(wtmp[:, 0:256],
                                  ins["Wr_out"][k * 128:(k + 1) * 128, :])
                nc.any.tensor_copy(Wr_bf[:, k, :], wtmp[:, 0:256])
                wtmp = wload.tile([128, 1024], f32, tag="wl")
                nc.sync.dma_start(wtmp[:, 0:256],
                                  ins["Wi_out"][k * 128:(k + 1) * 128, :])
                nc.any.tensor_copy(Wi_bf[:, k, :], wtmp[:, 0:256])
                nc.scalar.mul(Wineg_bf[:, k, :], wtmp[:, 0:256], -1.0)

            # ---- B: transpose seq (63 history + 64 action steps) to
            # xT [128(feat%128), KX, (t b)] bf16 via PE transposes
            xT = pre.tile([128, KX, NTB], bf16, name="xT")
            hist, act = ins["history"], ins["action"]
            npacks = (n_steps + 3) // 4
            for j in range(npacks):
                t0 = 4 * j
                nt = min(4, n_steps - t0)
                st = seqload.tile([128, 256], f32, tag="seq")
                # fill partitions [(tl*32):(tl*32+32)] with seq[t0+tl]
                runs = []  # (dst_row0, src_tensor, src_t0, cnt)
                tcur = t0
                while tcur < t0 + nt:
                    if tcur < 63:
                        cnt = min(63 - tcur, t0 + nt - tcur)
                        runs.append(((tcur - t0) * B, hist, tcur, cnt))
                    else:
                        cnt = t0 + nt - tcur
                        runs.append(((tcur - t0) * B, act, tcur - 63, cnt))
                    tcur += cnt
                for row0, src, s0, cnt in runs:
                    nc.sync.dma_start(
                        st[row0:row0 + cnt * B, :],
                        src[:, s0:s0 + cnt, :].rearrange("b t f -> t b f"))
                for fc in range(KX):
                    pt = ptr.tile([128, 128], f32, tag="tr")
                    nc.tensor.transpose(
                        pt[:, 0:nt * B],
                        st[0:nt * B, fc * 128:(fc + 1) * 128],
                        ident[0:nt * B, 0:nt * B])
                    nc.any.tensor_copy(
                        xT[:, fc, t0 * B:(t0 + nt) * B], pt[:, 0:nt * B])

            # ---- C: X[t] = Wx^T x_t + b_lstm, all steps -> XT (bf16)
            nsl = (n_steps * B + SL - 1) // SL
            for m in range(NM):
                XTm = XT[:, m, :, :].rearrange("p t b -> p (t b)")
                for ns in range(nsl):
                    c0 = ns * SL
                    c1 = min(c0 + SL, n_steps * B)
                    P = pz.tile([128, SL], f32, tag=("za" if ns % 2 == 0 else "zb"))
                    for k in range(KX):
                        nc.tensor.matmul(
                            P[:, 0:c1 - c0],
                            lhsT=Wx_bf[:, k, m * 128:(m + 1) * 128],
                            rhs=xT[:, k, c0:c1],
                            start=(k == 0), stop=(k == KX - 1))
                    if (m * nsl + ns) % 2 == 0:
                        nc.scalar.activation(XTm[:, c0:c1], P[:, 0:c1 - c0],
                                             AF.Identity, bias=b_lstm_t[:, m:m + 1])
                    else:
                        nc.vector.tensor_scalar_add(XTm[:, c0:c1], P[:, 0:c1 - c0],
                                                    b_lstm_t[:, m:m + 1])

        # ============ phase D: preamble MLP -> h0 = c0 = state^T ============
        with tc.tile_pool(name="dpool", bufs=1) as dpool, \
             tc.tile_pool(name="dload", bufs=2) as dload:
            # small weights -> bf16 (lhsT layout = natural [in, out])
            def _load_w(name, kparts, n):
                wt = dpool.tile([128, kparts, n], bf16, name=f"{name}_bf")
                for k in range(kparts):
                    wtmp = dload.tile([128, 512], f32, tag="dl")
                    nc.sync.dma_start(wtmp[:, 0:n],
                                      ins[name][k * 128:(k + 1) * 128, :])
                    nc.any.tensor_copy(wt[:, k, :], wtmp[:, 0:n])
                return wt

            Wros_bf = _load_w("W_ros", 1, 256)
            Wios_bf = _load_w("W_ios", 1, 256)
            Wms_bf = _load_w("W_ms", 1, 256)
            Wcos_bf = _load_w("W_cos", 4, 256)
            Wc_bf = _load_w("W_c", 6, 512)
            Wrs_bf = dpool.tile([128, 1, 256], bf16, name="W_rs_bf")
            wtmp = dload.tile([128, 512], f32, tag="dl")
            nc.sync.dma_start(wtmp[0:64, 0:256], ins["W_rs"][:, :])
            nc.any.tensor_copy(Wrs_bf[0:64, 0, :], wtmp[0:64, 0:256])

            # transpose the small state inputs to feature-on-partition bf16
            def _tr_in(name, rows):
                st = dload.tile([128, 512], f32, tag="dl")
                nc.sync.dma_start(st[0:B, 0:rows], ins[name][:, :])
                pt = ptr.tile([128, 128], f32, tag="tr")
                nc.tensor.transpose(pt[0:rows, 0:B], st[0:B, 0:rows],
                                    ident[0:B, 0:B])
                return pt

            p_mo = _tr_in("motion_state", 128)
            moT = dpool.tile([128, B], bf16, name="moT")
            nc.any.tensor_copy(moT[:, :], p_mo[:, 0:B])

            p_ro = _tr_in("robot_state", 64)
            roT = dpool.tile([128, B], bf16, name="roT")
            nc.any.tensor_copy(roT[0:64, :], p_ro[0:64, 0:B])

            reT = dpool.tile([128, B], bf16, name="reT")   # [osc_state_re; osc_re]
            imT = dpool.tile([128, B], bf16, name="imT")   # [osc_state_im; osc_im]
            p_os = _tr_in("osc_state", 128)
            nc.any.tensor_copy(reT[0:64, :], p_os[0:64, 0:B])
            nc.any.tensor_copy(imT[0:64, :], p_os[64:128, 0:B])
            p_oc = _tr_in("osc", 128)
            nc.any.tensor_copy(reT[64:128, :], p_oc[0:64, 0:B])
            nc.any.tensor_copy(imT[64:128, :], p_oc[64:128, 0:B])

            # stage 1: real_o / imag_o  [256 each -> 2 tiles of 128]
            P1 = pz.tile([128, 512], f32, tag="za")
            for m in range(2):
                nc.tensor.matmul(P1[:, m * B:(m + 1) * B],
                                 lhsT=Wros_bf[:, 0, m * 128:(m + 1) * 128],
                                 rhs=reT[:, :], start=True, stop=True)
            for m in range(2):
                nc.tensor.matmul(P1[:, (2 + m) * B:(3 + m) * B],
                                 lhsT=Wios_bf[:, 0, m * 128:(m + 1) * 128],
                                 rhs=imT[:, :], start=True, stop=True)
            ro_bf = dpool.tile([128, 2, B], bf16, name="ro_bf")
            io_bf = dpool.tile([128, 2, B], bf16, name="io_bf")
            for m in range(2):
                nc.scalar.activation(ro_bf[:, m, :], P1[:, m * B:(m + 1) * B],
                                     Tanh, bias=b_ros_t[:, m:m + 1])
                nc.scalar.activation(io_bf[:, m, :], P1[:, (2 + m) * B:(3 + m) * B],
                                     Tanh, bias=b_ios_t[:, m:m + 1])

            # stage 2: ms, rs, osc_s
            P2 = pz.tile([128, 512], f32, tag="za")
            for m in range(2):
                nc.tensor.matmul(P2[:, m * B:(m + 1) * B],
                                 lhsT=Wms_bf[:, 0, m * 128:(m + 1) * 128],
                                 rhs=moT[:, :], start=True, stop=True)
            for m in range(2):
                nc.tensor.matmul(P2[:, (2 + m) * B:(3 + m) * B],
                                 lhsT=Wrs_bf[0:64, 0, m * 128:(m + 1) * 128],
                                 rhs=roT[0:64, :], start=True, stop=True)
            cos_chunks = [ro_bf[:, 0, :], ro_bf[:, 1, :], io_bf[:, 0, :], io_bf[:, 1, :]]
            for m in range(2):
                for k in range(4):
                    nc.tensor.matmul(P2[:, (4 + m) * B:(5 + m) * B],
                                     lhsT=Wcos_bf[:, k, m * 128:(m + 1) * 128],
                                     rhs=cos_chunks[k],
                                     start=(k == 0), stop=(k == 3))
            ms_bf = dpool.tile([128, 2, B], bf16, name="ms_bf")
            rs_bf = dpool.tile([128, 2, B], bf16, name="rs_bf")
            os_bf = dpool.tile([128, 2, B], bf16, name="os_bf")
            for m in range(2):
                nc.scalar.activation(ms_bf[:, m, :], P2[:, m * B:(m + 1) * B],
                                     Tanh, bias=b_ms_t[:, m:m + 1])
                nc.scalar.activation(rs_bf[:, m, :], P2[:, (2 + m) * B:(3 + m) * B],
                                     Tanh, bias=b_rs_t[:, m:m + 1])
                nc.scalar.activation(os_bf[:, m, :], P2[:, (4 + m) * B:(5 + m) * B],
                                     Tanh, bias=b_cos_t[:, m:m + 1])

            # stage 3: state = tanh([ms rs osc_s] @ W_c + b_c)  -> h0 = c0
            P3 = pz.tile([128, 512], f32, tag="za")
            st_chunks = [ms_bf[:, 0, :], ms_bf[:, 1, :], rs_bf[:, 0, :],
                         rs_bf[:, 1, :], os_bf[:, 0, :], os_bf[:, 1, :]]
            for m in range(KH):
                for k in range(6):
                    nc.tensor.matmul(P3[:, m * B:(m + 1) * B],
                                     lhsT=Wc_bf[:, k, m * 128:(m + 1) * 128],
                                     rhs=st_chunks[k],
                                     start=(k == 0), stop=(k == 5))
            hT = rec.tile([128, KH, B], bf16, tag="h")
            for m in range(KH):
                nc.scalar.activation(c_st[:, m, :], P3[:, m * B:(m + 1) * B],
                                     Tanh, bias=b_c_t[:, m:m + 1])
            nc.vector.tensor_copy(hT[:, :, :], c_st[:, :, :])

        # ============ phase E: LSTM recurrence over n_steps ============
        for t in range(n_steps):
            Za = pz.tile([128, 12, B], f32, tag="za")   # i, f, g gates
            Zb = pz.tile([128, 4, B], f32, tag="zb")    # o gate
            for m in range(NM):
                dst = Za[:, m, :] if m < 12 else Zb[:, m - 12, :]
                for k in range(KH):
                    nc.tensor.matmul(dst,
                                     lhsT=Wh_bf[:, k, m * 128:(m + 1) * 128],
                                     rhs=hT[:, k, :],
                                     start=(k == 0), stop=(k == KH - 1))
            za_sb = rec.tile([128, 12, B], f32, tag="zsba")
            nc.vector.tensor_add(za_sb[:, :, :], Za[:, :, :], XT[:, 0:12, t, :])
            ga = rec.tile([128, 12, B], f32, tag="ga")
            nc.scalar.activation(ga[:, 0:8, :], za_sb[:, 0:8, :], Sig)
            nc.scalar.activation(ga[:, 8:12, :], za_sb[:, 8:12, :], Tanh)
            tmp = rec.tile([128, KH, B], f32, tag="tmp")
            nc.vector.tensor_mul(tmp[:, :, :], ga[:, 0:4, :], ga[:, 8:12, :])
            nc.vector.tensor_mul(c_st[:, :, :], ga[:, 4:8, :], c_st[:, :, :])
            nc.vector.tensor_add(c_st[:, :, :], c_st[:, :, :], tmp[:, :, :])
            zb_sb = rec.tile([128, 4, B], f32, tag="zsbb")
            nc.vector.tensor_add(zb_sb[:, :, :], Zb[:, :, :], XT[:, 12:16, t, :])
            gb = rec.tile([128, KH, B], f32, tag="gb")
            nc.scalar.activation(gb[:, :, :], zb_sb[:, :, :], Sig)
            tanh_c = rec.tile([128, KH, B], f32, tag="tanhc")
            nc.scalar.activation(tanh_c[:, :, :], c_st[:, :, :], Tanh)
            hT = rec.tile([128, KH, B], bf16, tag="h")
            nc.vector.tensor_mul(hT[:, :, :], gb[:, :, :], tanh_c[:, :, :])

        # ============ phase F: complex dense head + output transpose ========
        P4 = pz.tile([128, 4, B], f32, tag="zb")
        for m in range(2):
            # out_r tile m: Wr^T xr - Wi^T xi
            for k in range(2):
                nc.tensor.matmul(P4[:, m, :],
                                 lhsT=Wr_bf[:, k, m * 128:(m + 1) * 128],
                                 rhs=hT[:, k, :], start=(k == 0), stop=False)
            for k in range(2):
                nc.tensor.matmul(P4[:, m, :],
                                 lhsT=Wineg_bf[:, k, m * 128:(m + 1) * 128],
                                 rhs=hT[:, 2 + k, :], start=False, stop=(k == 1))
            # out_i tile m: Wi^T xr + Wr^T xi
            for k in range(2):
                nc.tensor.matmul(P4[:, 2 + m, :],
                                 lhsT=Wi_bf[:, k, m * 128:(m + 1) * 128],
                                 rhs=hT[:, k, :], start=(k == 0), stop=False)
            for k in range(2):
                nc.tensor.matmul(P4[:, 2 + m, :],
                                 lhsT=Wr_bf[:, k, m * 128:(m + 1) * 128],
                                 rhs=hT[:, 2 + k, :], start=False, stop=(k == 1))
        outT = rec.tile([128, 4, B], f32, tag="outT")
        for m in range(2):
            nc.scalar.activation(outT[:, m, :], P4[:, m, :], Tanh,
                                 bias=br_t[:, m:m + 1])
            nc.scalar.activation(outT[:, 2 + m, :], P4[:, 2 + m, :], Tanh,
                                 bias=bi_t[:, m:m + 1])
        out_sb = rec.tile([128, 512], f32, tag="out_sb")
        for j in range(4):
            pt = ptr.tile([128, 128], f32, tag="tr")
            nc.tensor.transpose(pt[0:B, 0:128], outT[:, j, :], ident[:, :])
            nc.any.tensor_copy(out_sb[0:B, j * 128:(j + 1) * 128], pt[0:B, 0:128])
        nc.sync.dma_start(outs["out"][:, :], out_sb[0:B, :])


_cached_nc = None


def _get_program():
    global _cached_nc
    if _cached_nc is None:
        nc = bacc.Bacc("TRN2", target_bir_lowering=False, debug=False)
        ins = {}
        for name, shape in SHAPES.items():
            ins[name] = nc.dram_tensor(name, list(shape), dt.float32,
                                       kind="ExternalInput")[...]
        out = nc.dram_tensor("out", [B, 512], dt.float32, kind="ExternalOutput")
        with tile.TileContext(nc) as tc:
            build(tc, {"out": out[...]}, ins)
        nc.finalize()  # bacc legalization (wait splitting, reg alloc, DCE)
        _cached_nc = nc
    return _cached_nc


def kernel(**inputs):
    nc = _get_program()
    in_maps = []
    for i in range(NCORES):
        m = {}
        for name in SHAPES:
            arr = np.ascontiguousarray(inputs[name], dtype=np.float32)
            if name in PER_BATCH:
                arr = np.ascontiguousarray(arr[i * B:(i + 1) * B])
            m[name] = arr
        in_maps.append(m)
    res = run_bass_kernel_spmd(nc, in_maps, list(range(NCORES)))
    return np.concatenate([res.results[i]["out"] for i in range(NCORES)], axis=0)


if __name__ == "__main__":
    import reference  # noqa: F401  (only for a local smoke run)
    inp = {k: np.asarray(v) for k, v in reference.setup_inputs().items()}
    out = kernel(**inp)
    print(out.shape, out.dtype)
